# revision 35
# baseline (speedup 1.0000x reference)
"""Local contrast normalization (9x9 Gaussian) Trainium2 Bass kernel.

Input x: [64, 512, 512, 1] f32. Output same shape:
    mean = conv2d_same(x, g9x9)
    d    = x - mean
    s    = conv2d_same(d*d, g9x9)
    norm = sqrt(s); keep = norm > 0.5
    out  = where(keep, d / norm, d)

Strategy (pure data parallel, 8 images per core on 8 cores):
  Each 512x512 image is processed in 5 row-windows of <=112 output rows.
  Window emission is software-pipelined (window w's conv work is emitted
  alongside window w-2's tail) so no engine queue head-of-line blocks,
  and input/output DMAs are paired across two images' same-index windows
  (one 3D-AP DMA each) with a one-pair prefetch lead.

  Each 9x9 conv: vertical taps fold into a banded stationary matrix,
  horizontal taps are free-dim offsets into a zero-margin fp8 SBUF tile.
  conv1 per window = 1 bf16 matmul carrying lam*(I - center-column band)
  on x_bf16 (PSUM = lam*d directly; x passthrough at bf16 precision)
  + 3 fp8 DoubleRow matmuls, each computing TWO horizontal tap columns
  (the K-planes are overlapping shifted views of the same fp8 x tile;
  the plane step MUST be even or the PE hangs - verified on HW).
  Columns {2,3,5,6,7} + a hi/lo correction plane for col 5; columns
  0,1,8 dropped (1.2% kernel mass, error-neutral vs fp8 noise).
  conv2 per window = 4 fp8 DoubleRow matmuls, columns 1..8, weights
  scaled by S2 and fp8-mass-compensated inside the rsqrt input scale.
  fp8 weights are scaled by lam/S2 to escape the e4m3 subnormal range;
  lam divides back out for free via activation/tensor_scalar scalars.

  Tail per window: Square (scalar -> fp8 dsq), Abs_reciprocal_sqrt
  (scalar -> bf16 r), mask = (r<2)/lam (vector), m1 = (r-1)*mask
  (vector), out = (m1 + 1/lam)*psum1 (vector, bf16 out).  Output DMA'd
  as bf16 (upcast on host).
"""

import sys

sys.path.insert(0, "/opt/trn_rl_repo")

import numpy as np

H = W = 512
IMGS_PER_CORE = 8
N_CORES = 8
CHUNK = 112
THRSHLD = 0.5
LAM = 256.0
S2 = 32.0

# conv1 DoubleRow pairs: (col, kind); plane step (colB-colA) must be EVEN
P1 = [
    ((3, "hi"), (5, "hi")),
    ((5, "lo"), (7, "hi")),
    ((2, "hi"), (6, "hi")),
]
# conv2 DoubleRow pairs: plain hi planes, even steps
P2 = [(1, 3), (5, 7), (2, 4), (6, 8)]

X8W = 520  # fp8 x tile width
DQW = 520  # dsq tile width


def _gauss2d():
    sigmah = 9 / 6.0
    ii = np.arange(9, dtype=np.float64)
    r2 = (ii[:, None] - 4.5) ** 2 + (ii[None, :] - 4.5) ** 2
    g = np.exp(-r2 / (2.0 * sigmah)).astype(np.float32)
    g = g / g.sum()
    return g.astype(np.float64)  # [dv, dj]


def _windows():
    out = []
    for c in range((H + CHUNK - 1) // CHUNK):
        O0, O1 = CHUNK * c, min(CHUNK * c + CHUNK, H)
        D0, D1 = max(0, O0 - 4), min(H, O1 + 4)
        X0, X1 = max(0, D0 - 4), min(H, D1 + 4)
        out.append((O0, O1, D0, D1, X0, X1))
    return out


WINDOWS = _windows()
N_WIN = len(WINDOWS)
T_OF = [0] + [1] * (N_WIN - 2) + [2]   # conv1 weight type per window
U_OF = [0] * (N_WIN - 1) + [1]         # conv2 weight type per window
T_GEOM = {0: (0, 120, 116), 1: (4, 128, 120), 2: (4, 72, 68)}  # xd, nX, nD
U_KLIM = {0: 128, 1: 68}


def _gen_weights():
    import ml_dtypes

    f8 = ml_dtypes.float8_e4m3
    bf = ml_dtypes.bfloat16
    g = _gauss2d()

    def q8(a):
        return np.asarray(a, np.float32).astype(f8).astype(np.float64)

    k = np.arange(128)[:, None]
    m = np.arange(128)[None, :]

    def band(gcol, xd, nX, nD):
        dv = k - m - xd + 4
        ok = (dv >= 0) & (dv <= 8) & (k < nX) & (m < nD)
        return np.where(ok, gcol[np.clip(dv, 0, 8)], 0.0)

    # conv1 bf16 center matrices: lam * (I - band(g[:,4]))
    W1c = np.zeros((3, 128, 128), np.float32)
    for t, (xd, nX, nD) in T_GEOM.items():
        ident = ((k - m == xd) & (k < nX) & (m < nD)).astype(np.float64)
        W1c[t] = (LAM * (ident - band(g[:, 4], xd, nX, nD))).astype(np.float32)
    W1c = W1c.astype(bf)

    # conv1 fp8 DoubleRow planes: -lam*g columns, hi/lo
    hi = {dj: q8(-LAM * g[:, dj]) for dj in [2, 3, 5, 6, 7]}
    lo = {5: q8(-LAM * g[:, 5] - hi[5])}
    W1p = np.zeros((3, len(P1), 2, 128, 128), np.float32)
    for t, (xd, nX, nD) in T_GEOM.items():
        for p, pair in enumerate(P1):
            for v, (dj, kind) in enumerate(pair):
                col = hi[dj] if kind == "hi" else lo[dj]
                W1p[t, p, v] = band(col, xd, nX, nD).astype(np.float32)
    W1p = W1p.astype(f8)

    # conv2 fp8 planes: S2*g columns, relative band in d-space
    W2p = np.zeros((2, 4, 2, 128, 128), np.float32)
    q2 = {dj: q8(S2 * g[:, dj]) for dj in range(1, 9)}
    for u, klim in U_KLIM.items():
        for p, (a, b) in enumerate(P2):
            W2p[u, p, 0] = band(q2[a], 0, klim, 128).astype(np.float32)
            W2p[u, p, 1] = band(q2[b], 0, klim, 128).astype(np.float32)
    W2p = W2p.astype(f8)

    # conv2 mass compensation (dropped col 0 + fp8 rounding) -> rsqrt scale
    qsum = sum((q2[dj] / S2).sum() for dj in range(1, 9))
    nu = g[:, 1:].sum() / qsum
    sigma = float(nu / S2)           # r = absrsqrt(sigma * psum2)
    return W1c, W1p, W2p, sigma


def _build_program():
    import concourse.bass as bass
    import concourse.bacc as bacc
    import concourse.tile as tile
    from concourse import mybir
    from concourse.ap import AP

    f32 = mybir.dt.float32
    bf16 = mybir.dt.bfloat16
    fp8 = mybir.dt.float8e4
    DR = mybir.MatmulPerfMode.DoubleRow

    _, _, _, sigma = _gen_weights()

    nc = bacc.Bacc("TRN2", target_bir_lowering=False, debug=False,
                   num_devices=N_CORES)

    rows = IMGS_PER_CORE * H
    x8_dram = nc.dram_tensor("x8", [rows, W], fp8, kind="ExternalInput")
    xb_dram = nc.dram_tensor("xb", [rows, W], bf16, kind="ExternalInput")
    w1c_dram = nc.dram_tensor("w1c", [128, 3 * 128], bf16, kind="ExternalInput")
    w1p_dram = nc.dram_tensor(
        "w1p", [128, 3 * len(P1) * 2 * 128], fp8, kind="ExternalInput"
    )
    w2p_dram = nc.dram_tensor("w2p", [128, 2 * 4 * 2 * 128], fp8, kind="ExternalInput")
    o_dram = nc.dram_tensor("out", [rows, W], bf16, kind="ExternalOutput")

    NSLOT = 4
    inv_lam = 1.0 / LAM

    with tile.TileContext(nc) as tc:
        with (
            tc.tile_pool(name="wpool", bufs=1) as wpool,
            tc.tile_pool(name="xpool", bufs=1) as xpool,
            tc.tile_pool(name="tpool", bufs=3) as tpool,
            tc.tile_pool(name="opool", bufs=3) as opool,
            tc.tile_pool(name="ps1", bufs=6, space=bass.MemorySpace.PSUM) as ps1,
            tc.tile_pool(name="ps2", bufs=2, space=bass.MemorySpace.PSUM) as ps2,
        ):
            w1c_sb = wpool.tile([128, 3, 128], bf16)
            w1p_sb = wpool.tile([128, 3, len(P1), 2, 128], fp8)
            w2p_sb = wpool.tile([128, 2, 4, 2, 128], fp8)
            # weight DMAs on the scalar HWDGE ring (low latency, queue idle
            # at start); gpsimd only runs the slot-init memsets, slot 0
            # first so square(0)'s dq WAR clears early.
            nc.scalar.dma_start(
                w1p_sb[:].rearrange("k t p v m -> k (t p v m)"), w1p_dram.ap()
            )
            nc.scalar.dma_start(
                w2p_sb[:].rearrange("k u p v m -> k (u p v m)"), w2p_dram.ap()
            )
            nc.scalar.dma_start(w1c_sb[:].rearrange("k t m -> k (t m)"), w1c_dram.ap())
            eps_sb = wpool.tile([128, 1], f32)
            nc.vector.memset(eps_sb[:], 1e-12)

            x8s, xbs, dqs = [], [], []
            for j in range(NSLOT):
                t8 = xpool.tile([128, 2, X8W], fp8, name=f"x8_{j}")
                tb = xpool.tile([128, 2, 512], bf16, name=f"xb_{j}")
                tq = xpool.tile([128, 2, DQW], fp8, name=f"dq_{j}")
                nc.gpsimd.memset(tq[:].rearrange("k i c -> k (i c)"), 0.0)
                nc.gpsimd.memset(t8[:].rearrange("k i c -> k (i c)"), 0.0)
                nc.vector.memset(tb[:].rearrange("k i c -> k (i c)"), 0.0)
                x8s.append(t8)
                xbs.append(tb)
                dqs.append(tq)

            # software-pipelined emission: window w's front half (DMA in,
            # conv1, square, conv2) is emitted together with window w-1's
            # tail (absrsqrt, mask, m1, out, DMA out).  This keeps each
            # engine queue free of head-of-line blocking: by the time the
            # scalar engine reaches absrsqrt(w-1), conv2(w-1) has long
            # retired behind conv1(w).
            wins = [
                (c, i)
                for c in range(N_WIN)
                for i in range(IMGS_PER_CORE)
            ]
            NW = len(wins)
            state = {}

            def prefetch(pidx):
                # issue pair pidx's input DMAs (one pair ahead of compute)
                widx = 2 * pidx
                if widx >= NW:
                    return
                c, i = wins[widx]
                O0, O1, D0, D1, X0, X1 = WINDOWS[c]
                nX = X1 - X0
                sl = pidx % NSLOT
                x8_t, xb_t = x8s[sl], xbs[sl]
                r0 = i * H + X0
                nc.sync.dma_start(
                    x8_t[0:nX, :, 4:516],
                    AP(x8_dram.ap().tensor, r0 * W,
                       [[W, nX], [H * W, 2], [1, 512]]),
                )
                nc.sync.dma_start(
                    xb_t[0:nX, :, :],
                    AP(xb_dram.ap().tensor, r0 * W,
                       [[W, nX], [H * W, 2], [1, 512]]),
                )

            def front(widx):
                c, i = wins[widx]
                O0, O1, D0, D1, X0, X1 = WINDOWS[c]
                nX, nD = X1 - X0, D1 - D0
                T, U = T_OF[c], U_OF[c]
                half = widx % 2
                sl = (widx // 2) % NSLOT
                x8_t, xb_t, dq_t = x8s[sl], xbs[sl], dqs[sl]
                if half == 0:
                    prefetch(widx // 2 + 1)
                psum1 = ps1.tile([128, 512], f32, tag="d")
                nc.tensor.matmul(
                    psum1[:], w1c_sb[:, T, :], xb_t[:, half, :],
                    start=True, stop=False,
                )
                for p, pair in enumerate(P1):
                    a, b = pair[0][0], pair[1][0]
                    nc.tensor.matmul(
                        psum1[:],
                        w1p_sb[:, T, p, :, :],
                        AP(x8_t[:].tensor, half * X8W + a,
                           [[2 * X8W, 128], [b - a, 2], [1, 512]]),
                        start=False, stop=(p == len(P1) - 1), perf_mode=DR,
                    )
                nc.scalar.activation(
                    dq_t[0:nD, half, 4:516], psum1[0:nD, :],
                    mybir.ActivationFunctionType.Square, scale=inv_lam,
                )
                psum2 = ps2.tile([128, 512], f32, tag="s")
                for p, (a, b) in enumerate(P2):
                    nc.tensor.matmul(
                        psum2[:],
                        w2p_sb[:, U, p, :, :],
                        AP(dq_t[:].tensor, half * DQW + a,
                           [[2 * DQW, 128], [b - a, 2], [1, 512]]),
                        start=(p == 0), stop=(p == 3), perf_mode=DR,
                    )
                state[widx] = (psum1, psum2)

            def tail(widx):
                c, i = wins[widx]
                O0, O1, D0, D1, X0, X1 = WINDOWS[c]
                nD, nO = D1 - D0, O1 - O0
                off2 = O0 - D0
                psum1, psum2 = state.pop(widx)
                r_t = tpool.tile([128, 512], bf16, tag="r")
                nc.scalar.activation(
                    r_t[0:nD, :], psum2[0:nD, :],
                    mybir.ActivationFunctionType.Abs_reciprocal_sqrt,
                    scale=sigma, bias=eps_sb[0:nD, :],
                )
                mk_t = tpool.tile([128, 512], bf16, tag="mask")
                nc.vector.tensor_scalar(
                    mk_t[0:nD, :], r_t[0:nD, :], 2.0, inv_lam,
                    mybir.AluOpType.is_lt, mybir.AluOpType.mult,
                )
                m1_t = tpool.tile([128, 512], bf16, tag="m1")
                nc.vector.scalar_tensor_tensor(
                    m1_t[0:nD, :], r_t[0:nD, :], 1.0, mk_t[0:nD, :],
                    mybir.AluOpType.subtract, mybir.AluOpType.mult,
                )
                half = widx % 2
                if half == 0:
                    o_t = opool.tile([128, 2, 512], bf16, tag="out")
                    state["o"] = o_t
                else:
                    o_t = state["o"]
                nc.vector.scalar_tensor_tensor(
                    o_t[0:nD, half, :], m1_t[0:nD, :], inv_lam, psum1[0:nD, :],
                    mybir.AluOpType.add, mybir.AluOpType.mult,
                )
                if widx >= NW - 2:
                    # final pair: ship each half immediately (shorter drain)
                    nc.scalar.dma_start(
                        o_dram.ap()[i * H + O0 : i * H + O1, :],
                        o_t[off2 : off2 + nO, half, :],
                    )
                elif half == 1:
                    nc.gpsimd.dma_start(
                        AP(o_dram.ap().tensor, ((i - 1) * H + O0) * W,
                           [[W, nO], [H * W, 2], [1, 512]]),
                        o_t[off2 : off2 + nO, :, :],
                    )

            prefetch(0)
            for w in range(NW + 1):
                if w < NW:
                    front(w)
                if w >= 1:
                    tail(w - 1)

    nc.compile()
    return nc


_NC = None


def _get_nc():
    global _NC
    if _NC is None:
        _NC = _build_program()
    return _NC


def _run(x_full, trace=False, **kw):
    from concourse import bass_utils
    import ml_dtypes

    f8 = ml_dtypes.float8_e4m3
    bf = ml_dtypes.bfloat16

    nc = _get_nc()
    W1c, W1p, W2p, _ = _gen_weights()
    w1c_h = np.ascontiguousarray(W1c.transpose(1, 0, 2).reshape(128, 3 * 128))
    w1p_h = np.ascontiguousarray(
        W1p.transpose(3, 0, 1, 2, 4).reshape(128, 3 * len(P1) * 2 * 128)
    )
    w2p_h = np.ascontiguousarray(
        W2p.transpose(3, 0, 1, 2, 4).reshape(128, 2 * 4 * 2 * 128)
    )

    x_full = np.asarray(x_full, dtype=np.float32).reshape(64, H, W)
    in_maps = []
    for core in range(N_CORES):
        shard = np.ascontiguousarray(
            x_full[core * IMGS_PER_CORE : (core + 1) * IMGS_PER_CORE].reshape(
                IMGS_PER_CORE * H, W
            )
        )
        in_maps.append(
            {
                "x8": shard.astype(f8),
                "xb": shard.astype(bf),
                "w1c": w1c_h,
                "w1p": w1p_h,
                "w2p": w2p_h,
            }
        )
    res = bass_utils.run_bass_kernel_spmd(
        nc, in_maps, core_ids=list(range(N_CORES)), trace=trace, **kw
    )
    out = np.concatenate(
        [
            np.asarray(r["out"]).astype(np.float32).reshape(IMGS_PER_CORE, H, W)
            for r in res.results
        ],
        axis=0,
    )
    return out.reshape(64, H, W, 1), res


def kernel(x):
    out, _ = _run(x)
    return out
